# revision 55
# baseline (speedup 1.0000x reference)
"""Trainium2 Bass kernel for MeshConv: SpMM (COO segment-sum) + Linear.

out[r] = (sum_e vals[e] * x[cols[e]] for rows[e]==r) @ W.T + b

Strategy (8 NeuronCores, pure data/graph parallel, bf16 internally):
  - 1D vertex partition of dest rows: core k owns 25088 rows as 196
    tiles x 128 slots.  Row->tile assignment is residue-aware (see
    below) and degree-balanced; x (bf16, [200704, 128]) is replicated.
  - Gathers use gpsimd.dma_gather (one instruction fetches thousands of
    256B rows).  Its int16 index limit is sidestepped by viewing x as
    [25088, 8*128]: residue r = col%8 selects one of 8 sliced base
    views, idx = col//8 <= 25087 fits int16.  One gather per
    (28-tile group, residue) => 56 gather ops/core instead of 1568
    indirect DMAs (994ns SWDGE fixed cost each).
  - Each (tile, residue) cell owns k columns of 128 edge slots; the
    fixed profile is k=1 plus a second column on residue tau%8 (K=9
    columns/tile).  A greedy assigner keeps every cell's edge count
    under its capacity, so the structure (and the compiled program) is
    input-independent.
  - Per tile: M[p, j] = (iota[j]==d[p])*v[p] built in 2 DVE ops; PE
    accumulates aggT[c, slot] += xg_col.T @ M_col over the 9 columns in
    f32 PSUM; ACT evacuates aggT to bf16; PE applies W (bf16); DVE adds
    bias; HWDGE stores the f32 tile.
"""
import sys

sys.path.insert(0, "/opt/trn_rl_repo")

import ml_dtypes
import numpy as np

import concourse.bass as bass
import concourse.mybir as mybir
from concourse.bacc import Bacc
from concourse.bass_utils import run_bass_kernel_spmd
from concourse.tile import TileContext

P = 128
C = 128
R = 8
N_VERTS = 200000
N_CORES = 8
TPC = 196  # tiles per core
N_PAD = N_CORES * TPC * P  # 200704
NR8 = N_PAD // R  # 25088 rows in the [NR8, 8*C] view of x
DUP = 4  # d/v image duplication (innermost DVE AP run length)
G_TILES = [28, 28, 28, 28, 28, 28, 16, 8, 4]  # tiles per gather group
assert sum(G_TILES) == TPC
N_GROUPS = len(G_TILES)
G_START = np.concatenate([[0], np.cumsum(G_TILES)]).astype(np.int64)
G_OF = np.repeat(np.arange(N_GROUPS), G_TILES)  # tile -> group

# Filled by kernel() when BASS_KERNEL_TRACE=1; read by test.py.
LAST_EXEC_NS = None
LAST_MEAN_EXEC_NS = None

_program_cache = {}


# --------------------------------------------------------------------------
# structure / layout derived from the per-cell column counts k[tau, r]
# --------------------------------------------------------------------------
def _profile_caps():
    caps = np.full((TPC, R), P, np.int32)
    caps[np.arange(TPC), np.arange(TPC) % R] = 2 * P
    return caps


class _Layout:
    """Region (g, r) = [28 regular columns (one per cell)] + [S[g,r] shared
    spill columns].  A tile's chain = its 8 regular columns + the spill
    columns of every region where P_spill[tau, r] (shared across cores)."""

    def __init__(self, S: np.ndarray, P_spill: np.ndarray, trim16: np.ndarray):
        self.S = S  # [N_GROUPS, R] spill columns per region
        self.P_spill = P_spill  # [TPC, R] bool: tile joins region spill
        self.trim16 = trim16  # [N_GROUPS, R] gathered spill idxs (16-mult)
        g_of = G_OF
        # columns per tile: 8 regular + spill columns of joined regions
        self.K_t = np.array(
            [
                R + sum(int(S[g_of[t], r]) for r in range(R) if P_spill[t, r])
                for t in range(TPC)
            ],
            np.int64,
        )
        self.Kmax = int(self.K_t.max())
        # d/v per-tile offsets (in columns; images store PAIR-duplicated
        # bf16 values, so the bf16 col offset of tile tau is 2*dvoff[tau])
        w = ((self.K_t + 1) // 2) * 2
        self.dvoff = np.zeros(TPC + 1, np.int64)
        self.dvoff[1:] = np.cumsum(w)
        self.dv_width = int(self.dvoff[-1])  # columns

        self.ncols = np.zeros((N_GROUPS, R), np.int64)  # cols per call
        self.regbase = np.zeros((N_GROUPS, R), np.int64)  # xg col base
        self.spillbase = np.zeros((N_GROUPS, R), np.int64)
        self.cellcol = {}  # (tau, r) -> regular xg col (within group)
        self.xgcol = np.zeros((TPC, self.Kmax), np.int64)  # tile col -> xg col
        self.jspill = {}  # (tau, r) -> first j of region-r spill in tile chain
        self.W_g = np.zeros(N_GROUPS, np.int64)
        for g in range(N_GROUPS):
            taus = range(int(G_START[g]), int(G_START[g + 1]))
            col = 0
            for r in range(R):
                self.regbase[g, r] = col
                for tau in taus:
                    self.cellcol[(tau, r)] = col
                    col += 1
                self.spillbase[g, r] = col
                col += int(S[g, r])
                self.ncols[g, r] = col - self.regbase[g, r]
            self.W_g[g] = col
        for tau in range(TPC):
            g = int(G_OF[tau])
            for r in range(R):
                self.xgcol[tau, r] = self.cellcol[(tau, r)]
            j = R
            for r in range(R):
                if self.P_spill[tau, r]:
                    self.jspill[(tau, r)] = j
                    for sc in range(int(S[g, r])):
                        self.xgcol[tau, j] = self.spillbase[g, r] + sc
                        j += 1
        self.Wmax = int(self.W_g.max())
        # idx stream: per group, per residue call of NI idxs (regular zone
        # fully + spill zone statically trimmed to trim16),
        # wrapped to [128, NI/16] int16; calls concatenated per group.
        self.NI = (
            np.array(G_TILES, np.int64)[:, None] * P + trim16.astype(np.int64)
        )
        self.idxoff16 = np.zeros((N_GROUPS, R + 1), np.int64)
        for g in range(N_GROUPS):
            for r in range(R):
                self.idxoff16[g, r + 1] = self.idxoff16[g, r] + self.NI[g, r] // 16
        self.IW_g = self.idxoff16[:, -1]  # int16 cols per group image
        self.IWmax = int(self.IW_g.max())


# --------------------------------------------------------------------------
# host: residue-aware degree-balanced row -> (core, tile, slot) assignment
# --------------------------------------------------------------------------
def _assign_rows(rows, cols):
    deg = np.bincount(rows, minlength=N_PAD).astype(np.int32)
    res = np.zeros((N_PAD, R), np.int32)
    np.add.at(res, (rows, cols % R), 1)

    order = np.argsort(-deg, kind="stable")
    snake = order.reshape(N_PAD // N_CORES, N_CORES).copy()
    snake[1::2] = snake[1::2, ::-1]

    caps = _profile_caps()
    soft = caps - 2
    big = np.iinfo(np.int64).max

    core_of_row = np.empty(N_PAD, np.int32)
    tloc_of_row = np.empty(N_PAD, np.int32)
    slot_of_row = np.empty(N_PAD, np.int32)
    loads = np.zeros((N_CORES, TPC, R), np.int32)

    for c in range(N_CORES):
        cr = snake[:, c]
        o = cr[np.argsort(-deg[cr], kind="stable")]
        load = np.zeros((TPC, R), np.int32)
        count = np.zeros(TPC, np.int32)
        for row in o:
            pr = res[row]
            new = load + pr
            over = np.clip(new - soft, 0, None)
            pen = (over * over).sum(axis=1)
            sec = (new * (pr > 0)).max(axis=1)
            cost = pen * 1000000 + sec
            cost[count >= P] = big
            t = int(np.argmin(cost))
            core_of_row[row] = c
            tloc_of_row[row] = t
            slot_of_row[row] = count[t]
            load[t] += pr
            count[t] += 1
        loads[c] = load
    return core_of_row, tloc_of_row, slot_of_row, loads


# --------------------------------------------------------------------------
# host: pack per-core idx stream and d/v images
# --------------------------------------------------------------------------
def _pack_core(lay, tau_e, r_e, idx_e, d_e, v_e):
    """tau_e/r_e/idx_e/d_e/v_e: per-edge arrays for one core."""
    cell = tau_e.astype(np.int64) * R + r_e
    order = np.argsort(cell, kind="stable")
    cell_s = cell[order]
    # rank within cell
    n = len(cell_s)
    starts = np.r_[0, np.flatnonzero(np.diff(cell_s)) + 1]
    rank = np.arange(n) - np.repeat(starts, np.diff(np.r_[starts, n]))
    tau_s = tau_e[order]
    r_s = r_e[order]
    g = G_OF[tau_s]

    cellcol = np.zeros((TPC, R), np.int64)
    for tau in range(TPC):
        for r in range(R):
            cellcol[tau, r] = lay.cellcol[(tau, r)]

    # regular slots (rank < P): slot p in the cell's own column, j = r
    gcol = cellcol[tau_s, r_s].copy()
    p = rank.copy()
    j = r_s.astype(np.int64).copy()

    # spill slots (rank >= P): packed sequentially per region (g, r),
    # ordered by (tau, rank) so each tile's spill stays contiguous
    sp = rank >= P
    if sp.any():
        region = g[sp] * R + r_s[sp]
        sord = np.argsort(region, kind="stable")  # tau,rank order preserved
        reg_s = region[sord]
        ns = len(reg_s)
        rstarts = np.r_[0, np.flatnonzero(np.diff(reg_s)) + 1]
        seq = np.arange(ns) - np.repeat(
            rstarts, np.diff(np.r_[rstarts, ns])
        )
        spill_idx = np.flatnonzero(sp)[sord]
        t_sp = tau_s[spill_idx]
        r_sp = r_s[spill_idx]
        g_sp = g[spill_idx]
        gcol[spill_idx] = lay.spillbase[g_sp, r_sp] + seq // P
        p[spill_idx] = seq % P
        jsp = np.array(
            [lay.jspill[(t, r)] for t, r in zip(t_sp, r_sp)], np.int64
        )
        j[spill_idx] = jsp + seq // P

    # ---- idx stream ----
    # Interior empty slots gather row 0 (the ucode has no per-idx negative
    # check -- negative interior idxs would read out of bounds).  Only the
    # TRAILING run of each call may be -1: the ucode trims it entirely.
    # position within call (g, r): (gcol - regbase)*128 + p
    pos_call = (gcol - lay.regbase[g, r_s]) * P + p
    # flat position within the whole per-core idx stream (pre-wrap):
    callstart = np.zeros((N_GROUPS, R), np.int64)
    acc = 0
    for gg in range(N_GROUPS):
        for r in range(R):
            callstart[gg, r] = acc
            acc += int(lay.NI[gg, r])
    flatpos = callstart[g, r_s] + pos_call
    stream = np.zeros(acc, np.int16)
    stream[flatpos] = idx_e[order].astype(np.int16)

    # wrap each call: [NI] -> [16, NI/16] -> tile x8 -> [128, NI/16]
    img_parts = []
    for gg in range(N_GROUPS):
        parts = []
        for r in range(R):
            s0 = callstart[gg, r]
            ni = int(lay.NI[gg, r])
            w = stream[s0 : s0 + ni].reshape(-1, 16).T  # [16, ni/16]
            parts.append(np.tile(w, (8, 1)))
        img_parts.append(np.concatenate(parts, axis=1))
    idx_img = np.concatenate(img_parts, axis=1)  # [128, sum IW_g]

    # ---- d / v images (DUP-duplicated: runs of DUP equal values so the
    # device is_equal/mult APs have innermost dim (DUP, step 1) -> DVE 2x
    # with few AP dimension transitions) ----
    dimg = np.full((P, lay.dv_width), -1.0, np.float32)
    vimg = np.zeros((P, lay.dv_width), np.float32)
    dvcol = lay.dvoff[tau_s] + j
    dimg[p, dvcol] = d_e[order].astype(np.float32)
    vimg[p, dvcol] = v_e[order]
    d_bf = np.repeat(dimg, DUP, axis=1).astype(ml_dtypes.bfloat16)
    v_bf = np.repeat(vimg, DUP, axis=1).astype(ml_dtypes.bfloat16)
    return idx_img, d_bf, v_bf


# --------------------------------------------------------------------------
# device program
# --------------------------------------------------------------------------
def _build_program(kkey: bytes):
    buf = np.frombuffer(kkey, np.int32)
    GR = N_GROUPS * R
    S = buf[:GR].reshape(N_GROUPS, R)
    trim16 = buf[GR : 2 * GR].reshape(N_GROUPS, R)
    P_spill = buf[2 * GR :].reshape(TPC, R).astype(bool)
    lay = _Layout(S, P_spill, trim16)
    f32 = mybir.dt.float32
    bf16 = mybir.dt.bfloat16
    i32 = mybir.dt.int32
    i16 = mybir.dt.int16
    nc = Bacc(num_swdge_queues=4)

    # consts (int32 words per partition):
    # [iota (Kmax*128 bf16) | wt (128 bf16) | bbT (1 f32) | d | v]
    # d/v are DUP-duplicated (DUP*dv_width bf16 cols each).
    iota_w = lay.Kmax * C // 2
    wt_w = C // 2
    bb_w = 1
    dv_w = lay.dv_width * DUP // 2  # int32 words per image
    o_iota = 0
    o_wt = o_iota + iota_w
    o_bb = o_wt + wt_w
    o_d = o_bb + bb_w
    o_v = o_d + dv_w
    CW = o_v + dv_w

    B4 = 4  # tiles per W-apply / store batch
    assert TPC % B4 == 0

    x8_d = nc.declare_dram_parameter("x8", [NR8, R * C], bf16, isOutput=False)
    idx_d = nc.declare_dram_parameter(
        "idx", [P, int(lay.IW_g.sum())], i16, isOutput=False
    )
    consts_d = nc.declare_dram_parameter("consts", [P, CW], i32, isOutput=False)
    out_d = nc.declare_dram_parameter(
        "out", [TPC // B4, C, B4 * P], f32, isOutput=True
    )

    idx_gbase = np.zeros(N_GROUPS + 1, np.int64)
    idx_gbase[1:] = np.cumsum(lay.IW_g)

    with TileContext(nc) as tc:
        with (
            tc.tile_pool(name="const", bufs=1) as cpool,
            tc.tile_pool(name="idx", bufs=2) as ipool,
            tc.tile_pool(name="xg", bufs=2) as xgpool,
            tc.tile_pool(name="msel", bufs=3) as mpool,
            tc.tile_pool(name="evac", bufs=4) as epool,
            tc.tile_pool(name="outs", bufs=4) as opool,
            tc.tile_pool(name="ps_agg", bufs=6, space="PSUM") as pa_pool,
            tc.tile_pool(name="ps_out", bufs=2, space="PSUM") as po_pool,
        ):
            consts_s = cpool.tile([P, CW], i32)
            nc.sync.dma_start(out=consts_s[:], in_=consts_d[:])
            cbf = consts_s[:].bitcast(bf16)
            iota_s = cbf[:, 2 * o_iota : 2 * o_iota + lay.Kmax * C]
            wt_s = cbf[:, 2 * o_wt : 2 * o_wt + C]
            bbT_s = consts_s[:, o_bb : o_bb + 1].bitcast(f32)
            d_all = cbf[:, 2 * o_d : 2 * o_d + DUP * lay.dv_width]
            v_all = cbf[:, 2 * o_v : 2 * o_v + DUP * lay.dv_width]

            for g in range(N_GROUPS):
                iw = int(lay.IW_g[g])
                idx_t = ipool.tile([P, lay.IWmax], i16, tag="idx")
                nc.sync.dma_start(
                    out=idx_t[:, :iw],
                    in_=idx_d[:, int(idx_gbase[g]) : int(idx_gbase[g]) + iw],
                )
                wg = int(lay.W_g[g])
                xg_t = xgpool.tile([P, lay.Wmax * C], bf16, tag="xg")
                xg = xg_t[:, : wg * C]
                if g < 2:
                    # zero the spill columns of both ring buffers once:
                    # their trailing slots are never gathered (trimmed -1
                    # tail) and must hold finite bf16 so M=0 kills them
                    for r in range(R):
                        sb = int(lay.spillbase[g, r])
                        ns = int(lay.S[g, r])
                        if ns:
                            nc.scalar.memzero(
                                xg[:, sb * C : (sb + ns) * C]
                            )
                for r in range(R):
                    a = int(lay.regbase[g, r])
                    ni = int(lay.NI[g, r])
                    ncol = (ni + P - 1) // P
                    if ncol == 0:
                        continue
                    i0 = int(lay.idxoff16[g, r])
                    nc.gpsimd.dma_gather(
                        xg[:, a * C : (a + ncol) * C].rearrange(
                            "p (k c) -> p k c", k=ncol
                        ),
                        x8_d[:, r * C : (r + 1) * C],
                        idx_t[:, i0 : i0 + ni // 16],
                        ni,
                        ni,
                        C,
                        elem_step=R * C,
                        single_packet=False,
                        queue_num=r % 4,
                    )
                for bt in range(int(G_START[g]) // B4, int(G_START[g + 1]) // B4):
                    ag = epool.tile([P, B4 * P], bf16, tag="ag")
                    for i in range(B4):
                        tau = bt * B4 + i
                        K = int(lay.K_t[tau])
                        dv0 = int(lay.dvoff[tau])
                        m = mpool.tile([P, K * C], bf16, tag="m")
                        # innermost (DUP, step 1) on all operands -> DVE 2x
                        m4 = m[:].rearrange(
                            "p (j y t) -> p j y t", j=K, y=C // DUP, t=DUP
                        )
                        d2 = (
                            d_all[:, DUP * dv0 : DUP * (dv0 + K)]
                            .rearrange("p (j t) -> p j t", t=DUP)
                            .unsqueeze(2)
                            .broadcast_to([P, K, C // DUP, DUP])
                        )
                        v2 = (
                            v_all[:, DUP * dv0 : DUP * (dv0 + K)]
                            .rearrange("p (j t) -> p j t", t=DUP)
                            .unsqueeze(2)
                            .broadcast_to([P, K, C // DUP, DUP])
                        )
                        nc.vector.tensor_tensor(
                            out=m4,
                            in0=iota_s[:, : K * C].rearrange(
                                "p (j y t) -> p j y t", j=K, y=C // DUP, t=DUP
                            ),
                            in1=d2,
                            op=mybir.AluOpType.is_equal,
                        )
                        nc.vector.tensor_tensor(
                            out=m4,
                            in0=m4,
                            in1=v2,
                            op=mybir.AluOpType.mult,
                        )
                        ps = pa_pool.tile([P, P], f32, tag="ps_agg")
                        for j in range(K):
                            xc = int(lay.xgcol[tau, j])
                            nc.tensor.matmul(
                                out=ps[:],
                                lhsT=xg[:, xc * C : (xc + 1) * C],
                                rhs=m[:, j * C : (j + 1) * C],
                                start=(j == 0),
                                stop=(j == K - 1),
                            )
                        nc.scalar.copy(
                            out=ag[:, i * P : (i + 1) * P], in_=ps[:]
                        )
                    # poT[o, (i d)] = sum_c W.T[c, o] * agg.T[c, (i d)]
                    po = po_pool.tile([P, B4 * P], f32, tag="ps_out")
                    nc.tensor.matmul(
                        out=po[:], lhsT=wt_s, rhs=ag[:], start=True, stop=True
                    )
                    ot = opool.tile([P, B4 * P], f32, tag="outs")
                    nc.scalar.add(out=ot[:], in_=po[:], add=bbT_s)
                    nc.sync.dma_start(out=out_d[bt], in_=ot[:])

    nc.compile()
    return nc, lay


# --------------------------------------------------------------------------
# entry point
# --------------------------------------------------------------------------
def kernel(x, rows, cols, vals, W, b):
    global LAST_EXEC_NS, LAST_MEAN_EXEC_NS
    import os

    x = np.ascontiguousarray(np.asarray(x), dtype=np.float32)
    rows = np.asarray(rows).astype(np.int64, copy=False)
    cols = np.asarray(cols).astype(np.int64, copy=False)
    vals = np.asarray(vals).astype(np.float32, copy=False)
    W = np.asarray(W).astype(np.float32, copy=False)
    b = np.asarray(b).astype(np.float32, copy=False)

    core_of_row, tloc_of_row, slot_of_row, loads = _assign_rows(rows, cols)

    # shared structure: spill columns per region + spill participation,
    # maxed/or'd across cores so one program serves all 8
    over = np.clip(loads - P, 0, None)  # [cores, TPC, R]
    spill_reg = np.stack(
        [
            over[:, G_START[g] : G_START[g + 1], :].sum(axis=1)
            for g in range(N_GROUPS)
        ],
        axis=1,
    )  # [cores, G, R]
    spill_max = spill_reg.max(axis=0)  # [G, R] worst core
    S = np.ceil(spill_max / P).astype(np.int32)
    trim16 = ((spill_max + 15) // 16 * 16).astype(np.int32)
    P_spill = (over.max(axis=0) > 0).astype(np.int32)  # [TPC, R]
    kkey = (
        np.concatenate([S.ravel(), trim16.ravel(), P_spill.ravel()])
        .astype(np.int32)
        .tobytes()
    )
    if kkey not in _program_cache:
        _program_cache[kkey] = _build_program(kkey)
    nc, lay = _program_cache[kkey]

    # ---- pack inputs ----
    xb = np.zeros((N_PAD, C), ml_dtypes.bfloat16)
    xb[:N_VERTS] = x.astype(ml_dtypes.bfloat16)
    x8 = np.ascontiguousarray(xb.reshape(NR8, R * C))

    iota = np.tile(np.arange(C, dtype=np.float32), lay.Kmax).astype(
        ml_dtypes.bfloat16
    )
    iota_img = np.broadcast_to(iota, (P, lay.Kmax * C))
    wt = np.ascontiguousarray(W.T.astype(ml_dtypes.bfloat16))  # [c, o]
    wt_img = np.zeros((P, C), ml_dtypes.bfloat16)
    wt_img[:, :] = wt
    bbT_img = np.ascontiguousarray(b[:, None].astype(np.float32))  # [o, 1]

    e_core = core_of_row[rows]
    tau_e_all = tloc_of_row[rows]
    d_e_all = slot_of_row[rows]
    r_e_all = (cols % R).astype(np.int32)
    idx_e_all = (cols // R).astype(np.int32)

    in_maps = []
    for c in range(N_CORES):
        sel = e_core == c
        idx_img, d_bf, v_bf = _pack_core(
            lay,
            tau_e_all[sel],
            r_e_all[sel],
            idx_e_all[sel],
            d_e_all[sel],
            vals[sel],
        )
        consts = np.concatenate(
            [
                np.ascontiguousarray(iota_img).view(np.int32),
                wt_img.view(np.int32),
                bbT_img.view(np.int32),
                d_bf.view(np.int32),
                v_bf.view(np.int32),
            ],
            axis=1,
        )
        in_maps.append(
            {"x8": x8, "idx": np.ascontiguousarray(idx_img), "consts": consts}
        )

    trace = bool(os.environ.get("BASS_KERNEL_TRACE"))
    res = run_bass_kernel_spmd(nc, in_maps, list(range(N_CORES)), trace=trace)
    LAST_EXEC_NS = getattr(res, "exec_time_ns", None)
    LAST_MEAN_EXEC_NS = getattr(res, "mean_exec_time_ns", None)

    outs = [
        np.asarray(res.results[i]["out"])  # [TPC//4, C, 4*P] transposed tiles
        .reshape(TPC // 4, C, 4, P)
        .transpose(0, 2, 3, 1)
        .reshape(TPC * P, C)
        for i in range(N_CORES)
    ]
    full = np.concatenate(outs, axis=0)  # [N_PAD, C] in permuted order
    row_position = (
        core_of_row.astype(np.int64) * (TPC * P)
        + tloc_of_row.astype(np.int64) * P
        + slot_of_row
    )
    return np.ascontiguousarray(full[row_position[:N_VERTS]], dtype=np.float32)



# revision 56
# speedup vs baseline: 1.0435x; 1.0435x over previous
"""Trainium2 Bass kernel for MeshConv: SpMM (COO segment-sum) + Linear.

out[r] = (sum_e vals[e] * x[cols[e]] for rows[e]==r) @ W.T + b

Strategy (8 NeuronCores, pure data/graph parallel, bf16 internally):
  - 1D vertex partition of dest rows: core k owns 25088 rows as 196
    tiles x 128 slots.  Row->tile assignment is residue-aware (see
    below) and degree-balanced; x (bf16, [200704, 128]) is replicated.
  - Gathers use gpsimd.dma_gather (one instruction fetches thousands of
    256B rows).  Its int16 index limit is sidestepped by viewing x as
    [25088, 8*128]: residue r = col%8 selects one of 8 sliced base
    views, idx = col//8 <= 25087 fits int16.  One gather per
    (28-tile group, residue) => 56 gather ops/core instead of 1568
    indirect DMAs (994ns SWDGE fixed cost each).
  - Each (tile, residue) cell owns k columns of 128 edge slots; the
    fixed profile is k=1 plus a second column on residue tau%8 (K=9
    columns/tile).  A greedy assigner keeps every cell's edge count
    under its capacity, so the structure (and the compiled program) is
    input-independent.
  - Per tile: M[p, j] = (iota[j]==d[p])*v[p] built in 2 DVE ops; PE
    accumulates aggT[c, slot] += xg_col.T @ M_col over the 9 columns in
    f32 PSUM; ACT evacuates aggT to bf16; PE applies W (bf16); DVE adds
    bias; HWDGE stores the f32 tile.
"""
import sys

sys.path.insert(0, "/opt/trn_rl_repo")

import ml_dtypes
import numpy as np

import concourse.bass as bass
import concourse.mybir as mybir
from concourse.bacc import Bacc
from concourse.bass_utils import run_bass_kernel_spmd
from concourse.tile import TileContext

P = 128
C = 128
R = 8
N_VERTS = 200000
N_CORES = 8
TPC = 196  # tiles per core
N_PAD = N_CORES * TPC * P  # 200704
NR8 = N_PAD // R  # 25088 rows in the [NR8, 8*C] view of x
G_TILES = [28, 28, 28, 28, 28, 28, 16, 8, 4]  # tiles per gather group
assert sum(G_TILES) == TPC
N_GROUPS = len(G_TILES)
G_START = np.concatenate([[0], np.cumsum(G_TILES)]).astype(np.int64)
G_OF = np.repeat(np.arange(N_GROUPS), G_TILES)  # tile -> group

# Filled by kernel() when BASS_KERNEL_TRACE=1; read by test.py.
LAST_EXEC_NS = None
LAST_MEAN_EXEC_NS = None

_program_cache = {}


# --------------------------------------------------------------------------
# structure / layout derived from the per-cell column counts k[tau, r]
# --------------------------------------------------------------------------
def _profile_caps():
    caps = np.full((TPC, R), P, np.int32)
    caps[np.arange(TPC), np.arange(TPC) % R] = 2 * P
    return caps


class _Layout:
    """Region (g, r) = [28 regular columns (one per cell)] + [S[g,r] shared
    spill columns].  A tile's chain = its 8 regular columns + the spill
    columns of every region where P_spill[tau, r] (shared across cores)."""

    def __init__(self, S: np.ndarray, P_spill: np.ndarray, trim16: np.ndarray):
        self.S = S  # [N_GROUPS, R] spill columns per region
        self.P_spill = P_spill  # [TPC, R] bool: tile joins region spill
        self.trim16 = trim16  # [N_GROUPS, R] gathered spill idxs (16-mult)
        g_of = G_OF
        # columns per tile: 8 regular + spill columns of joined regions
        self.K_t = np.array(
            [
                R + sum(int(S[g_of[t], r]) for r in range(R) if P_spill[t, r])
                for t in range(TPC)
            ],
            np.int64,
        )
        self.Kmax = int(self.K_t.max())
        # d/v per-tile offsets (in columns; images store PAIR-duplicated
        # bf16 values, so the bf16 col offset of tile tau is 2*dvoff[tau])
        w = ((self.K_t + 1) // 2) * 2
        self.dvoff = np.zeros(TPC + 1, np.int64)
        self.dvoff[1:] = np.cumsum(w)
        self.dv_width = int(self.dvoff[-1])  # columns

        self.ncols = np.zeros((N_GROUPS, R), np.int64)  # cols per call
        self.regbase = np.zeros((N_GROUPS, R), np.int64)  # xg col base
        self.spillbase = np.zeros((N_GROUPS, R), np.int64)
        self.cellcol = {}  # (tau, r) -> regular xg col (within group)
        self.xgcol = np.zeros((TPC, self.Kmax), np.int64)  # tile col -> xg col
        self.jspill = {}  # (tau, r) -> first j of region-r spill in tile chain
        self.W_g = np.zeros(N_GROUPS, np.int64)
        for g in range(N_GROUPS):
            taus = range(int(G_START[g]), int(G_START[g + 1]))
            col = 0
            for r in range(R):
                self.regbase[g, r] = col
                for tau in taus:
                    self.cellcol[(tau, r)] = col
                    col += 1
                self.spillbase[g, r] = col
                col += int(S[g, r])
                self.ncols[g, r] = col - self.regbase[g, r]
            self.W_g[g] = col
        for tau in range(TPC):
            g = int(G_OF[tau])
            for r in range(R):
                self.xgcol[tau, r] = self.cellcol[(tau, r)]
            j = R
            for r in range(R):
                if self.P_spill[tau, r]:
                    self.jspill[(tau, r)] = j
                    for sc in range(int(S[g, r])):
                        self.xgcol[tau, j] = self.spillbase[g, r] + sc
                        j += 1
        self.Wmax = int(self.W_g.max())
        # idx stream: per group, per residue call of NI idxs (regular zone
        # fully + spill zone statically trimmed to trim16),
        # wrapped to [128, NI/16] int16; calls concatenated per group.
        self.NI = (
            np.array(G_TILES, np.int64)[:, None] * P + trim16.astype(np.int64)
        )
        self.idxoff16 = np.zeros((N_GROUPS, R + 1), np.int64)
        for g in range(N_GROUPS):
            for r in range(R):
                self.idxoff16[g, r + 1] = self.idxoff16[g, r] + self.NI[g, r] // 16
        self.IW_g = self.idxoff16[:, -1]  # int16 cols per group image
        self.IWmax = int(self.IW_g.max())


# --------------------------------------------------------------------------
# host: residue-aware degree-balanced row -> (core, tile, slot) assignment
# --------------------------------------------------------------------------
def _assign_rows(rows, cols):
    deg = np.bincount(rows, minlength=N_PAD).astype(np.int32)
    res = np.zeros((N_PAD, R), np.int32)
    np.add.at(res, (rows, cols % R), 1)

    order = np.argsort(-deg, kind="stable")
    snake = order.reshape(N_PAD // N_CORES, N_CORES).copy()
    snake[1::2] = snake[1::2, ::-1]

    caps = _profile_caps()
    soft = caps - 2
    big = np.iinfo(np.int64).max

    core_of_row = np.empty(N_PAD, np.int32)
    tloc_of_row = np.empty(N_PAD, np.int32)
    slot_of_row = np.empty(N_PAD, np.int32)
    loads = np.zeros((N_CORES, TPC, R), np.int32)

    for c in range(N_CORES):
        cr = snake[:, c]
        o = cr[np.argsort(-deg[cr], kind="stable")]
        load = np.zeros((TPC, R), np.int32)
        count = np.zeros(TPC, np.int32)
        for row in o:
            pr = res[row]
            new = load + pr
            over = np.clip(new - soft, 0, None)
            pen = (over * over).sum(axis=1)
            sec = (new * (pr > 0)).max(axis=1)
            cost = pen * 1000000 + sec
            cost[count >= P] = big
            t = int(np.argmin(cost))
            core_of_row[row] = c
            tloc_of_row[row] = t
            slot_of_row[row] = count[t]
            load[t] += pr
            count[t] += 1
        loads[c] = load
    return core_of_row, tloc_of_row, slot_of_row, loads


# --------------------------------------------------------------------------
# host: pack per-core idx stream and d/v images
# --------------------------------------------------------------------------
def _pack_core(lay, tau_e, r_e, idx_e, d_e, v_e):
    """tau_e/r_e/idx_e/d_e/v_e: per-edge arrays for one core."""
    cell = tau_e.astype(np.int64) * R + r_e
    order = np.argsort(cell, kind="stable")
    cell_s = cell[order]
    # rank within cell
    n = len(cell_s)
    starts = np.r_[0, np.flatnonzero(np.diff(cell_s)) + 1]
    rank = np.arange(n) - np.repeat(starts, np.diff(np.r_[starts, n]))
    tau_s = tau_e[order]
    r_s = r_e[order]
    g = G_OF[tau_s]

    cellcol = np.zeros((TPC, R), np.int64)
    for tau in range(TPC):
        for r in range(R):
            cellcol[tau, r] = lay.cellcol[(tau, r)]

    # regular slots (rank < P): slot p in the cell's own column, j = r
    gcol = cellcol[tau_s, r_s].copy()
    p = rank.copy()
    j = r_s.astype(np.int64).copy()

    # spill slots (rank >= P): packed sequentially per region (g, r),
    # ordered by (tau, rank) so each tile's spill stays contiguous
    sp = rank >= P
    if sp.any():
        region = g[sp] * R + r_s[sp]
        sord = np.argsort(region, kind="stable")  # tau,rank order preserved
        reg_s = region[sord]
        ns = len(reg_s)
        rstarts = np.r_[0, np.flatnonzero(np.diff(reg_s)) + 1]
        seq = np.arange(ns) - np.repeat(
            rstarts, np.diff(np.r_[rstarts, ns])
        )
        spill_idx = np.flatnonzero(sp)[sord]
        t_sp = tau_s[spill_idx]
        r_sp = r_s[spill_idx]
        g_sp = g[spill_idx]
        gcol[spill_idx] = lay.spillbase[g_sp, r_sp] + seq // P
        p[spill_idx] = seq % P
        jsp = np.array(
            [lay.jspill[(t, r)] for t, r in zip(t_sp, r_sp)], np.int64
        )
        j[spill_idx] = jsp + seq // P

    # ---- idx stream ----
    # Interior empty slots gather row 0 (the ucode has no per-idx negative
    # check -- negative interior idxs would read out of bounds).  Only the
    # TRAILING run of each call may be -1: the ucode trims it entirely.
    # position within call (g, r): (gcol - regbase)*128 + p
    pos_call = (gcol - lay.regbase[g, r_s]) * P + p
    # flat position within the whole per-core idx stream (pre-wrap):
    callstart = np.zeros((N_GROUPS, R), np.int64)
    acc = 0
    for gg in range(N_GROUPS):
        for r in range(R):
            callstart[gg, r] = acc
            acc += int(lay.NI[gg, r])
    flatpos = callstart[g, r_s] + pos_call
    stream = np.zeros(acc, np.int16)
    stream[flatpos] = idx_e[order].astype(np.int16)

    # wrap each call: [NI] -> [16, NI/16] -> tile x8 -> [128, NI/16]
    img_parts = []
    for gg in range(N_GROUPS):
        parts = []
        for r in range(R):
            s0 = callstart[gg, r]
            ni = int(lay.NI[gg, r])
            w = stream[s0 : s0 + ni].reshape(-1, 16).T  # [16, ni/16]
            parts.append(np.tile(w, (8, 1)))
        img_parts.append(np.concatenate(parts, axis=1))
    idx_img = np.concatenate(img_parts, axis=1)  # [128, sum IW_g]

    # ---- d / v images (pair-duplicated: value at cols 2k and 2k+1 so the
    # device is_equal/mult APs have innermost dim (2, step 1) -> DVE 2x) ----
    dimg = np.full((P, lay.dv_width), -1.0, np.float32)
    vimg = np.zeros((P, lay.dv_width), np.float32)
    dvcol = lay.dvoff[tau_s] + j
    dimg[p, dvcol] = d_e[order].astype(np.float32)
    vimg[p, dvcol] = v_e[order]
    d_bf = np.repeat(dimg, 2, axis=1).astype(ml_dtypes.bfloat16)
    v_bf = np.repeat(vimg, 2, axis=1).astype(ml_dtypes.bfloat16)
    return idx_img, d_bf, v_bf


# --------------------------------------------------------------------------
# device program
# --------------------------------------------------------------------------
def _build_program(kkey: bytes):
    buf = np.frombuffer(kkey, np.int32)
    GR = N_GROUPS * R
    S = buf[:GR].reshape(N_GROUPS, R)
    trim16 = buf[GR : 2 * GR].reshape(N_GROUPS, R)
    P_spill = buf[2 * GR :].reshape(TPC, R).astype(bool)
    lay = _Layout(S, P_spill, trim16)
    f32 = mybir.dt.float32
    bf16 = mybir.dt.bfloat16
    i32 = mybir.dt.int32
    i16 = mybir.dt.int16
    nc = Bacc(num_swdge_queues=4)

    # consts (int32 words per partition):
    # [iota (Kmax*128 bf16) | wt (128 bf16) | bbT (1 f32) | d2 | v2]
    # d2/v2 are pair-duplicated (2*dv_width bf16 cols each).
    iota_w = lay.Kmax * C // 2
    wt_w = C // 2
    bb_w = 1
    dv_w = lay.dv_width  # 2*dv_width bf16 = dv_width int32 words
    o_iota = 0
    o_wt = o_iota + iota_w
    o_bb = o_wt + wt_w
    o_d = o_bb + bb_w
    o_v = o_d + dv_w
    CW = o_v + dv_w

    B4 = 4  # tiles per W-apply / store batch
    assert TPC % B4 == 0

    x8_d = nc.declare_dram_parameter("x8", [NR8, R * C], bf16, isOutput=False)
    idx_d = nc.declare_dram_parameter(
        "idx", [P, int(lay.IW_g.sum())], i16, isOutput=False
    )
    consts_d = nc.declare_dram_parameter("consts", [P, CW], i32, isOutput=False)
    out_d = nc.declare_dram_parameter(
        "out", [TPC // B4, C, B4 * P], f32, isOutput=True
    )

    idx_gbase = np.zeros(N_GROUPS + 1, np.int64)
    idx_gbase[1:] = np.cumsum(lay.IW_g)

    with TileContext(nc) as tc:
        with (
            tc.tile_pool(name="const", bufs=1) as cpool,
            tc.tile_pool(name="idx", bufs=1) as ipool,
            tc.tile_pool(name="xg", bufs=2) as xgpool,
            tc.tile_pool(name="msel", bufs=3) as mpool,
            tc.tile_pool(name="evac", bufs=4) as epool,
            tc.tile_pool(name="outs", bufs=4) as opool,
            tc.tile_pool(name="ps_agg", bufs=6, space="PSUM") as pa_pool,
            tc.tile_pool(name="ps_out", bufs=2, space="PSUM") as po_pool,
        ):
            # group 0's idx slice first so the first gather isn't gated on
            # the whole image
            idx_all = ipool.tile([P, int(lay.IW_g.sum())], i16)
            iw0 = int(lay.IW_g[0])
            nc.sync.dma_start(out=idx_all[:, :iw0], in_=idx_d[:, :iw0])
            nc.sync.dma_start(out=idx_all[:, iw0:], in_=idx_d[:, iw0:])
            consts_s = cpool.tile([P, CW], i32)
            nc.sync.dma_start(out=consts_s[:], in_=consts_d[:])
            cbf = consts_s[:].bitcast(bf16)
            iota_s = cbf[:, 2 * o_iota : 2 * o_iota + lay.Kmax * C]
            wt_s = cbf[:, 2 * o_wt : 2 * o_wt + C]
            bbT_s = consts_s[:, o_bb : o_bb + 1].bitcast(f32)
            d_all = cbf[:, 2 * o_d : 2 * o_d + 2 * lay.dv_width]
            v_all = cbf[:, 2 * o_v : 2 * o_v + 2 * lay.dv_width]

            for g in range(N_GROUPS):
                wg = int(lay.W_g[g])
                xg_t = xgpool.tile([P, lay.Wmax * C], bf16, tag="xg")
                xg = xg_t[:, : wg * C]
                if g < 2:
                    # zero the spill columns of both ring buffers once:
                    # their trailing slots are never gathered (trimmed -1
                    # tail) and must hold finite bf16 so M=0 kills them
                    for r in range(R):
                        sb = int(lay.spillbase[g, r])
                        ns = int(lay.S[g, r])
                        if ns:
                            nc.scalar.memzero(
                                xg[:, sb * C : (sb + ns) * C]
                            )
                for r in range(R):
                    a = int(lay.regbase[g, r])
                    ni = int(lay.NI[g, r])
                    ncol = (ni + P - 1) // P
                    if ncol == 0:
                        continue
                    i0 = int(idx_gbase[g] + lay.idxoff16[g, r])
                    nc.gpsimd.dma_gather(
                        xg[:, a * C : (a + ncol) * C].rearrange(
                            "p (k c) -> p k c", k=ncol
                        ),
                        x8_d[:, r * C : (r + 1) * C],
                        idx_all[:, i0 : i0 + ni // 16],
                        ni,
                        ni,
                        C,
                        elem_step=R * C,
                        single_packet=False,
                        queue_num=r % 4,
                    )
                for bt in range(int(G_START[g]) // B4, int(G_START[g + 1]) // B4):
                    ag = epool.tile([P, B4 * P], bf16, tag="ag")
                    for i in range(B4):
                        tau = bt * B4 + i
                        K = int(lay.K_t[tau])
                        dv0 = int(lay.dvoff[tau])
                        m = mpool.tile([P, K * C], bf16, tag="m")
                        # innermost (2, step 1) on all operands -> DVE 2x
                        m4 = m[:].rearrange(
                            "p (j y t) -> p j y t", j=K, y=C // 2, t=2
                        )
                        d2 = (
                            d_all[:, 2 * dv0 : 2 * dv0 + 2 * K]
                            .rearrange("p (j t) -> p j t", t=2)
                            .unsqueeze(2)
                            .broadcast_to([P, K, C // 2, 2])
                        )
                        v2 = (
                            v_all[:, 2 * dv0 : 2 * dv0 + 2 * K]
                            .rearrange("p (j t) -> p j t", t=2)
                            .unsqueeze(2)
                            .broadcast_to([P, K, C // 2, 2])
                        )
                        nc.vector.tensor_tensor(
                            out=m4,
                            in0=iota_s[:, : K * C].rearrange(
                                "p (j y t) -> p j y t", j=K, y=C // 2, t=2
                            ),
                            in1=d2,
                            op=mybir.AluOpType.is_equal,
                        )
                        nc.vector.tensor_tensor(
                            out=m4,
                            in0=m4,
                            in1=v2,
                            op=mybir.AluOpType.mult,
                        )
                        ps = pa_pool.tile([P, P], f32, tag="ps_agg")
                        for j in range(K):
                            xc = int(lay.xgcol[tau, j])
                            nc.tensor.matmul(
                                out=ps[:],
                                lhsT=xg[:, xc * C : (xc + 1) * C],
                                rhs=m[:, j * C : (j + 1) * C],
                                start=(j == 0),
                                stop=(j == K - 1),
                            )
                        nc.scalar.copy(
                            out=ag[:, i * P : (i + 1) * P], in_=ps[:]
                        )
                    # poT[o, (i d)] = sum_c W.T[c, o] * agg.T[c, (i d)]
                    po = po_pool.tile([P, B4 * P], f32, tag="ps_out")
                    nc.tensor.matmul(
                        out=po[:], lhsT=wt_s, rhs=ag[:], start=True, stop=True
                    )
                    ot = opool.tile([P, B4 * P], f32, tag="outs")
                    nc.scalar.add(out=ot[:], in_=po[:], add=bbT_s)
                    nc.sync.dma_start(out=out_d[bt], in_=ot[:])

    nc.compile()
    return nc, lay


# --------------------------------------------------------------------------
# entry point
# --------------------------------------------------------------------------
def kernel(x, rows, cols, vals, W, b):
    global LAST_EXEC_NS, LAST_MEAN_EXEC_NS
    import os

    x = np.ascontiguousarray(np.asarray(x), dtype=np.float32)
    rows = np.asarray(rows).astype(np.int64, copy=False)
    cols = np.asarray(cols).astype(np.int64, copy=False)
    vals = np.asarray(vals).astype(np.float32, copy=False)
    W = np.asarray(W).astype(np.float32, copy=False)
    b = np.asarray(b).astype(np.float32, copy=False)

    core_of_row, tloc_of_row, slot_of_row, loads = _assign_rows(rows, cols)

    # shared structure: spill columns per region + spill participation,
    # maxed/or'd across cores so one program serves all 8
    over = np.clip(loads - P, 0, None)  # [cores, TPC, R]
    spill_reg = np.stack(
        [
            over[:, G_START[g] : G_START[g + 1], :].sum(axis=1)
            for g in range(N_GROUPS)
        ],
        axis=1,
    )  # [cores, G, R]
    spill_max = spill_reg.max(axis=0)  # [G, R] worst core
    S = np.ceil(spill_max / P).astype(np.int32)
    trim16 = ((spill_max + 15) // 16 * 16).astype(np.int32)
    P_spill = (over.max(axis=0) > 0).astype(np.int32)  # [TPC, R]
    kkey = (
        np.concatenate([S.ravel(), trim16.ravel(), P_spill.ravel()])
        .astype(np.int32)
        .tobytes()
    )
    if kkey not in _program_cache:
        _program_cache[kkey] = _build_program(kkey)
    nc, lay = _program_cache[kkey]

    # ---- pack inputs ----
    xb = np.zeros((N_PAD, C), ml_dtypes.bfloat16)
    xb[:N_VERTS] = x.astype(ml_dtypes.bfloat16)
    x8 = np.ascontiguousarray(xb.reshape(NR8, R * C))

    iota = np.tile(np.arange(C, dtype=np.float32), lay.Kmax).astype(
        ml_dtypes.bfloat16
    )
    iota_img = np.broadcast_to(iota, (P, lay.Kmax * C))
    wt = np.ascontiguousarray(W.T.astype(ml_dtypes.bfloat16))  # [c, o]
    wt_img = np.zeros((P, C), ml_dtypes.bfloat16)
    wt_img[:, :] = wt
    bbT_img = np.ascontiguousarray(b[:, None].astype(np.float32))  # [o, 1]

    e_core = core_of_row[rows]
    tau_e_all = tloc_of_row[rows]
    d_e_all = slot_of_row[rows]
    r_e_all = (cols % R).astype(np.int32)
    idx_e_all = (cols // R).astype(np.int32)

    in_maps = []
    for c in range(N_CORES):
        sel = e_core == c
        idx_img, d_bf, v_bf = _pack_core(
            lay,
            tau_e_all[sel],
            r_e_all[sel],
            idx_e_all[sel],
            d_e_all[sel],
            vals[sel],
        )
        consts = np.concatenate(
            [
                np.ascontiguousarray(iota_img).view(np.int32),
                wt_img.view(np.int32),
                bbT_img.view(np.int32),
                d_bf.view(np.int32),
                v_bf.view(np.int32),
            ],
            axis=1,
        )
        in_maps.append(
            {"x8": x8, "idx": np.ascontiguousarray(idx_img), "consts": consts}
        )

    trace = bool(os.environ.get("BASS_KERNEL_TRACE"))
    res = run_bass_kernel_spmd(nc, in_maps, list(range(N_CORES)), trace=trace)
    LAST_EXEC_NS = getattr(res, "exec_time_ns", None)
    LAST_MEAN_EXEC_NS = getattr(res, "mean_exec_time_ns", None)

    outs = [
        np.asarray(res.results[i]["out"])  # [TPC//4, C, 4*P] transposed tiles
        .reshape(TPC // 4, C, 4, P)
        .transpose(0, 2, 3, 1)
        .reshape(TPC * P, C)
        for i in range(N_CORES)
    ]
    full = np.concatenate(outs, axis=0)  # [N_PAD, C] in permuted order
    row_position = (
        core_of_row.astype(np.int64) * (TPC * P)
        + tloc_of_row.astype(np.int64) * P
        + slot_of_row
    )
    return np.ascontiguousarray(full[row_position[:N_VERTS]], dtype=np.float32)



# revision 59
# speedup vs baseline: 1.0530x; 1.0092x over previous
"""Trainium2 Bass kernel for MeshConv: SpMM (COO segment-sum) + Linear.

out[r] = (sum_e vals[e] * x[cols[e]] for rows[e]==r) @ W.T + b

Strategy (8 NeuronCores, pure data/graph parallel, bf16 internally):
  - 1D vertex partition of dest rows: core k owns 25088 rows as 196
    tiles x 128 slots.  Row->tile assignment is residue-aware and
    degree-balanced; x (bf16, [200704, 128]) is replicated.
  - Gathers use gpsimd.dma_gather (one instruction fetches thousands of
    256B rows).  Its int16 index limit is sidestepped by viewing x as
    [25088, 8*128]: residue r = col%8 selects one of 8 sliced base
    views, idx = col//8 <= 25087 fits int16.  One gather per
    (group, residue); gathers round-robin over the 4 SWDGE queues
    (num_swdge_queues=4) so all 8 Q7 cores generate descriptors in
    parallel -- Q7 descriptor generation is the kernel's bottleneck.
  - Groups are uneven ([28x6,16,8,4]) so the final group's gather+
    compute tail is short.  Each (tile, residue) cell owns one regular
    column of 128 edge slots; per-cell overflow goes to a single
    shared spill column per (group, residue), statically trimmed to
    the worst core's usage (NI = tiles*128 + trim16).
  - Per tile: M[p, j] = (iota[j]==d[p])*v[p] built in 2 DVE ops whose
    operands use pair-duplicated d/v images so every AP has innermost
    (2, step 1) -> DVE 2x mode; PE accumulates aggT[c, slot] +=
    xg_col.T @ M_col over the ~9 columns in f32 PSUM; ACT evacuates
    aggT to bf16; one PE matmul per 4 tiles applies W transposed
    (poT[o, 4*128] = W.T' @ ag-batch); ACT adds bias (per-partition)
    during PSUM evac; HWDGE stores f32 [C, 4*128] tiles that the host
    un-transposes.
"""
import sys

sys.path.insert(0, "/opt/trn_rl_repo")

import ml_dtypes
import numpy as np

import concourse.bass as bass
import concourse.mybir as mybir
from concourse.bacc import Bacc
from concourse.bass_utils import run_bass_kernel_spmd
from concourse.tile import TileContext

P = 128
C = 128
R = 8
N_VERTS = 200000
N_CORES = 8
TPC = 196  # tiles per core
N_PAD = N_CORES * TPC * P  # 200704
NR8 = N_PAD // R  # 25088 rows in the [NR8, 8*C] view of x
G_TILES = [28, 28, 28, 28, 28, 28, 20, 8]  # tiles per gather group
assert sum(G_TILES) == TPC
N_GROUPS = len(G_TILES)
G_START = np.concatenate([[0], np.cumsum(G_TILES)]).astype(np.int64)
G_OF = np.repeat(np.arange(N_GROUPS), G_TILES)  # tile -> group

# Filled by kernel() when BASS_KERNEL_TRACE=1; read by test.py.
LAST_EXEC_NS = None
LAST_MEAN_EXEC_NS = None

_program_cache = {}


# --------------------------------------------------------------------------
# structure / layout derived from the per-cell column counts k[tau, r]
# --------------------------------------------------------------------------
def _profile_caps():
    caps = np.full((TPC, R), P, np.int32)
    caps[np.arange(TPC), np.arange(TPC) % R] = 2 * P
    return caps


class _Layout:
    """Region (g, r) = [28 regular columns (one per cell)] + [S[g,r] shared
    spill columns].  A tile's chain = its 8 regular columns + the spill
    columns of every region where P_spill[tau, r] (shared across cores)."""

    def __init__(self, S: np.ndarray, P_spill: np.ndarray, trim16: np.ndarray):
        self.S = S  # [N_GROUPS, R] spill columns per region
        self.P_spill = P_spill  # [TPC, R] bool: tile joins region spill
        self.trim16 = trim16  # [N_GROUPS, R] gathered spill idxs (16-mult)
        g_of = G_OF
        # columns per tile: 8 regular + spill columns of joined regions
        self.K_t = np.array(
            [
                R + sum(int(S[g_of[t], r]) for r in range(R) if P_spill[t, r])
                for t in range(TPC)
            ],
            np.int64,
        )
        self.Kmax = int(self.K_t.max())
        # d/v per-tile offsets (in columns; images store PAIR-duplicated
        # bf16 values, so the bf16 col offset of tile tau is 2*dvoff[tau])
        w = ((self.K_t + 1) // 2) * 2
        self.dvoff = np.zeros(TPC + 1, np.int64)
        self.dvoff[1:] = np.cumsum(w)
        self.dv_width = int(self.dvoff[-1])  # columns

        self.ncols = np.zeros((N_GROUPS, R), np.int64)  # cols per call
        self.regbase = np.zeros((N_GROUPS, R), np.int64)  # xg col base
        self.spillbase = np.zeros((N_GROUPS, R), np.int64)
        self.cellcol = {}  # (tau, r) -> regular xg col (within group)
        self.xgcol = np.zeros((TPC, self.Kmax), np.int64)  # tile col -> xg col
        self.jspill = {}  # (tau, r) -> first j of region-r spill in tile chain
        self.W_g = np.zeros(N_GROUPS, np.int64)
        for g in range(N_GROUPS):
            taus = range(int(G_START[g]), int(G_START[g + 1]))
            col = 0
            for r in range(R):
                self.regbase[g, r] = col
                for tau in taus:
                    self.cellcol[(tau, r)] = col
                    col += 1
                self.spillbase[g, r] = col
                col += int(S[g, r])
                self.ncols[g, r] = col - self.regbase[g, r]
            self.W_g[g] = col
        for tau in range(TPC):
            g = int(G_OF[tau])
            for r in range(R):
                self.xgcol[tau, r] = self.cellcol[(tau, r)]
            j = R
            for r in range(R):
                if self.P_spill[tau, r]:
                    self.jspill[(tau, r)] = j
                    for sc in range(int(S[g, r])):
                        self.xgcol[tau, j] = self.spillbase[g, r] + sc
                        j += 1
        self.Wmax = int(self.W_g.max())
        # idx stream: per group, per residue call of NI idxs (regular zone
        # fully + spill zone statically trimmed to trim16),
        # wrapped to [128, NI/16] int16; calls concatenated per group.
        self.NI = (
            np.array(G_TILES, np.int64)[:, None] * P + trim16.astype(np.int64)
        )
        self.idxoff16 = np.zeros((N_GROUPS, R + 1), np.int64)
        for g in range(N_GROUPS):
            for r in range(R):
                self.idxoff16[g, r + 1] = self.idxoff16[g, r] + self.NI[g, r] // 16
        self.IW_g = self.idxoff16[:, -1]  # int16 cols per group image
        self.IWmax = int(self.IW_g.max())


# --------------------------------------------------------------------------
# host: residue-aware degree-balanced row -> (core, tile, slot) assignment
# --------------------------------------------------------------------------
def _assign_rows(rows, cols):
    deg = np.bincount(rows, minlength=N_PAD).astype(np.int32)
    res = np.zeros((N_PAD, R), np.int32)
    np.add.at(res, (rows, cols % R), 1)

    order = np.argsort(-deg, kind="stable")
    snake = order.reshape(N_PAD // N_CORES, N_CORES).copy()
    snake[1::2] = snake[1::2, ::-1]

    caps = _profile_caps()
    soft = caps - 2
    big = np.iinfo(np.int64).max

    core_of_row = np.empty(N_PAD, np.int32)
    tloc_of_row = np.empty(N_PAD, np.int32)
    slot_of_row = np.empty(N_PAD, np.int32)
    loads = np.zeros((N_CORES, TPC, R), np.int32)

    for c in range(N_CORES):
        cr = snake[:, c]
        o = cr[np.argsort(-deg[cr], kind="stable")]
        load = np.zeros((TPC, R), np.int32)
        count = np.zeros(TPC, np.int32)
        for row in o:
            pr = res[row]
            new = load + pr
            over = np.clip(new - soft, 0, None)
            pen = (over * over).sum(axis=1)
            sec = (new * (pr > 0)).max(axis=1)
            cost = pen * 1000000 + sec
            cost[count >= P] = big
            t = int(np.argmin(cost))
            core_of_row[row] = c
            tloc_of_row[row] = t
            slot_of_row[row] = count[t]
            load[t] += pr
            count[t] += 1
        loads[c] = load
    return core_of_row, tloc_of_row, slot_of_row, loads


# --------------------------------------------------------------------------
# host: pack per-core idx stream and d/v images
# --------------------------------------------------------------------------
def _pack_core(lay, tau_e, r_e, idx_e, d_e, v_e):
    """tau_e/r_e/idx_e/d_e/v_e: per-edge arrays for one core."""
    cell = tau_e.astype(np.int64) * R + r_e
    order = np.argsort(cell, kind="stable")
    cell_s = cell[order]
    # rank within cell
    n = len(cell_s)
    starts = np.r_[0, np.flatnonzero(np.diff(cell_s)) + 1]
    rank = np.arange(n) - np.repeat(starts, np.diff(np.r_[starts, n]))
    tau_s = tau_e[order]
    r_s = r_e[order]
    g = G_OF[tau_s]

    cellcol = np.zeros((TPC, R), np.int64)
    for tau in range(TPC):
        for r in range(R):
            cellcol[tau, r] = lay.cellcol[(tau, r)]

    # regular slots (rank < P): slot p in the cell's own column, j = r
    gcol = cellcol[tau_s, r_s].copy()
    p = rank.copy()
    j = r_s.astype(np.int64).copy()

    # spill slots (rank >= P): packed sequentially per region (g, r),
    # ordered by (tau, rank) so each tile's spill stays contiguous
    sp = rank >= P
    if sp.any():
        region = g[sp] * R + r_s[sp]
        sord = np.argsort(region, kind="stable")  # tau,rank order preserved
        reg_s = region[sord]
        ns = len(reg_s)
        rstarts = np.r_[0, np.flatnonzero(np.diff(reg_s)) + 1]
        seq = np.arange(ns) - np.repeat(
            rstarts, np.diff(np.r_[rstarts, ns])
        )
        spill_idx = np.flatnonzero(sp)[sord]
        t_sp = tau_s[spill_idx]
        r_sp = r_s[spill_idx]
        g_sp = g[spill_idx]
        gcol[spill_idx] = lay.spillbase[g_sp, r_sp] + seq // P
        p[spill_idx] = seq % P
        jsp = np.array(
            [lay.jspill[(t, r)] for t, r in zip(t_sp, r_sp)], np.int64
        )
        j[spill_idx] = jsp + seq // P

    # ---- idx stream ----
    # Interior empty slots gather row 0 (the ucode has no per-idx negative
    # check -- negative interior idxs would read out of bounds).  Only the
    # TRAILING run of each call may be -1: the ucode trims it entirely.
    # position within call (g, r): (gcol - regbase)*128 + p
    pos_call = (gcol - lay.regbase[g, r_s]) * P + p
    # flat position within the whole per-core idx stream (pre-wrap):
    callstart = np.zeros((N_GROUPS, R), np.int64)
    acc = 0
    for gg in range(N_GROUPS):
        for r in range(R):
            callstart[gg, r] = acc
            acc += int(lay.NI[gg, r])
    flatpos = callstart[g, r_s] + pos_call
    stream = np.zeros(acc, np.int16)
    stream[flatpos] = idx_e[order].astype(np.int16)

    # wrap each call: [NI] -> [16, NI/16] -> tile x8 -> [128, NI/16]
    img_parts = []
    for gg in range(N_GROUPS):
        parts = []
        for r in range(R):
            s0 = callstart[gg, r]
            ni = int(lay.NI[gg, r])
            w = stream[s0 : s0 + ni].reshape(-1, 16).T  # [16, ni/16]
            parts.append(np.tile(w, (8, 1)))
        img_parts.append(np.concatenate(parts, axis=1))
    idx_img = np.concatenate(img_parts, axis=1)  # [128, sum IW_g]

    # ---- d / v images (pair-duplicated: value at cols 2k and 2k+1 so the
    # device is_equal/mult APs have innermost dim (2, step 1) -> DVE 2x) ----
    dimg = np.full((P, lay.dv_width), -1.0, np.float32)
    vimg = np.zeros((P, lay.dv_width), np.float32)
    dvcol = lay.dvoff[tau_s] + j
    dimg[p, dvcol] = d_e[order].astype(np.float32)
    vimg[p, dvcol] = v_e[order]
    d_bf = np.repeat(dimg, 2, axis=1).astype(ml_dtypes.bfloat16)
    v_bf = np.repeat(vimg, 2, axis=1).astype(ml_dtypes.bfloat16)
    return idx_img, d_bf, v_bf


# --------------------------------------------------------------------------
# device program
# --------------------------------------------------------------------------
def _build_program(kkey: bytes):
    buf = np.frombuffer(kkey, np.int32)
    GR = N_GROUPS * R
    S = buf[:GR].reshape(N_GROUPS, R)
    trim16 = buf[GR : 2 * GR].reshape(N_GROUPS, R)
    P_spill = buf[2 * GR :].reshape(TPC, R).astype(bool)
    lay = _Layout(S, P_spill, trim16)
    f32 = mybir.dt.float32
    bf16 = mybir.dt.bfloat16
    i32 = mybir.dt.int32
    i16 = mybir.dt.int16
    nc = Bacc(num_swdge_queues=4)

    # consts (int32 words per partition):
    # [iota (Kmax*128 bf16) | wt (128 bf16) | bbT (1 f32) | d2 | v2]
    # d2/v2 are pair-duplicated (2*dv_width bf16 cols each).
    iota_w = lay.Kmax * C // 2
    wt_w = C // 2
    bb_w = 1
    dv_w = lay.dv_width  # 2*dv_width bf16 = dv_width int32 words
    o_iota = 0
    o_wt = o_iota + iota_w
    o_bb = o_wt + wt_w
    o_d = o_bb + bb_w
    o_v = o_d + dv_w
    CW = o_v + dv_w

    B4 = 4  # tiles per W-apply / store batch
    assert TPC % B4 == 0

    x8_d = nc.declare_dram_parameter("x8", [NR8, R * C], bf16, isOutput=False)
    idx_d = nc.declare_dram_parameter(
        "idx", [P, int(lay.IW_g.sum())], i16, isOutput=False
    )
    consts_d = nc.declare_dram_parameter("consts", [P, CW], i32, isOutput=False)
    out_d = nc.declare_dram_parameter(
        "out", [TPC // B4, C, B4 * P], f32, isOutput=True
    )

    idx_gbase = np.zeros(N_GROUPS + 1, np.int64)
    idx_gbase[1:] = np.cumsum(lay.IW_g)

    with TileContext(nc) as tc:
        with (
            tc.tile_pool(name="const", bufs=1) as cpool,
            tc.tile_pool(name="idx", bufs=1) as ipool,
            tc.tile_pool(name="xg", bufs=2) as xgpool,
            tc.tile_pool(name="msel", bufs=2) as mpool,
            tc.tile_pool(name="evac", bufs=4) as epool,
            tc.tile_pool(name="outs", bufs=4) as opool,
            tc.tile_pool(name="ps_agg", bufs=6, space="PSUM") as pa_pool,
            tc.tile_pool(name="ps_out", bufs=2, space="PSUM") as po_pool,
        ):
            # group 0's idx slice first so the first gather isn't gated on
            # the whole image
            idx_all = ipool.tile([P, int(lay.IW_g.sum())], i16)
            iw0 = int(lay.IW_g[0])
            nc.sync.dma_start(out=idx_all[:, :iw0], in_=idx_d[:, :iw0])
            nc.sync.dma_start(out=idx_all[:, iw0:], in_=idx_d[:, iw0:])
            consts_s = cpool.tile([P, CW], i32)
            nc.sync.dma_start(out=consts_s[:], in_=consts_d[:])
            cbf = consts_s[:].bitcast(bf16)
            iota_s = cbf[:, 2 * o_iota : 2 * o_iota + lay.Kmax * C]
            wt_s = cbf[:, 2 * o_wt : 2 * o_wt + C]
            bbT_s = consts_s[:, o_bb : o_bb + 1].bitcast(f32)
            d_all = cbf[:, 2 * o_d : 2 * o_d + 2 * lay.dv_width]
            v_all = cbf[:, 2 * o_v : 2 * o_v + 2 * lay.dv_width]

            for g in range(N_GROUPS):
                wg = int(lay.W_g[g])
                xg_t = xgpool.tile([P, lay.Wmax * C], bf16, tag="xg")
                xg = xg_t[:, : wg * C]
                if g < 2:
                    # zero the spill columns of both ring buffers once:
                    # their trailing slots are never gathered (trimmed -1
                    # tail) and must hold finite bf16 so M=0 kills them
                    for r in range(R):
                        sb = int(lay.spillbase[g, r])
                        ns = int(lay.S[g, r])
                        if ns:
                            nc.scalar.memzero(
                                xg[:, sb * C : (sb + ns) * C]
                            )
                for r in range(R):
                    a = int(lay.regbase[g, r])
                    ni = int(lay.NI[g, r])
                    ncol = (ni + P - 1) // P
                    if ncol == 0:
                        continue
                    i0 = int(idx_gbase[g] + lay.idxoff16[g, r])
                    nc.gpsimd.dma_gather(
                        xg[:, a * C : (a + ncol) * C].rearrange(
                            "p (k c) -> p k c", k=ncol
                        ),
                        x8_d[:, r * C : (r + 1) * C],
                        idx_all[:, i0 : i0 + ni // 16],
                        ni,
                        ni,
                        C,
                        elem_step=R * C,
                        single_packet=False,
                        queue_num=r % 4,
                    )
                for bt in range(int(G_START[g]) // B4, int(G_START[g + 1]) // B4):
                    ag = epool.tile([P, B4 * P], bf16, tag="ag")
                    for i in range(B4):
                        tau = bt * B4 + i
                        K = int(lay.K_t[tau])
                        dv0 = int(lay.dvoff[tau])
                        m = mpool.tile([P, K * C], bf16, tag="m")
                        # innermost (2, step 1) on all operands -> DVE 2x
                        m4 = m[:].rearrange(
                            "p (j y t) -> p j y t", j=K, y=C // 2, t=2
                        )
                        d2 = (
                            d_all[:, 2 * dv0 : 2 * dv0 + 2 * K]
                            .rearrange("p (j t) -> p j t", t=2)
                            .unsqueeze(2)
                            .broadcast_to([P, K, C // 2, 2])
                        )
                        v2 = (
                            v_all[:, 2 * dv0 : 2 * dv0 + 2 * K]
                            .rearrange("p (j t) -> p j t", t=2)
                            .unsqueeze(2)
                            .broadcast_to([P, K, C // 2, 2])
                        )
                        nc.vector.tensor_tensor(
                            out=m4,
                            in0=iota_s[:, : K * C].rearrange(
                                "p (j y t) -> p j y t", j=K, y=C // 2, t=2
                            ),
                            in1=d2,
                            op=mybir.AluOpType.is_equal,
                        )
                        nc.vector.tensor_tensor(
                            out=m4,
                            in0=m4,
                            in1=v2,
                            op=mybir.AluOpType.mult,
                        )
                        ps = pa_pool.tile([P, P], f32, tag="ps_agg")
                        for j in range(K):
                            xc = int(lay.xgcol[tau, j])
                            nc.tensor.matmul(
                                out=ps[:],
                                lhsT=xg[:, xc * C : (xc + 1) * C],
                                rhs=m[:, j * C : (j + 1) * C],
                                start=(j == 0),
                                stop=(j == K - 1),
                            )
                        nc.scalar.copy(
                            out=ag[:, i * P : (i + 1) * P], in_=ps[:]
                        )
                    # poT[o, (i d)] = sum_c W.T[c, o] * agg.T[c, (i d)]
                    po = po_pool.tile([P, B4 * P], f32, tag="ps_out")
                    nc.tensor.matmul(
                        out=po[:], lhsT=wt_s, rhs=ag[:], start=True, stop=True
                    )
                    ot = opool.tile([P, B4 * P], f32, tag="outs")
                    nc.scalar.add(out=ot[:], in_=po[:], add=bbT_s)
                    nc.sync.dma_start(out=out_d[bt], in_=ot[:])

    nc.compile()
    return nc, lay


# --------------------------------------------------------------------------
# entry point
# --------------------------------------------------------------------------
def kernel(x, rows, cols, vals, W, b):
    global LAST_EXEC_NS, LAST_MEAN_EXEC_NS
    import os

    x = np.ascontiguousarray(np.asarray(x), dtype=np.float32)
    rows = np.asarray(rows).astype(np.int64, copy=False)
    cols = np.asarray(cols).astype(np.int64, copy=False)
    vals = np.asarray(vals).astype(np.float32, copy=False)
    W = np.asarray(W).astype(np.float32, copy=False)
    b = np.asarray(b).astype(np.float32, copy=False)

    core_of_row, tloc_of_row, slot_of_row, loads = _assign_rows(rows, cols)

    # shared structure: spill columns per region + spill participation,
    # maxed/or'd across cores so one program serves all 8
    over = np.clip(loads - P, 0, None)  # [cores, TPC, R]
    spill_reg = np.stack(
        [
            over[:, G_START[g] : G_START[g + 1], :].sum(axis=1)
            for g in range(N_GROUPS)
        ],
        axis=1,
    )  # [cores, G, R]
    spill_max = spill_reg.max(axis=0)  # [G, R] worst core
    S = np.ceil(spill_max / P).astype(np.int32)
    trim16 = ((spill_max + 15) // 16 * 16).astype(np.int32)
    P_spill = (over.max(axis=0) > 0).astype(np.int32)  # [TPC, R]
    kkey = (
        np.concatenate([S.ravel(), trim16.ravel(), P_spill.ravel()])
        .astype(np.int32)
        .tobytes()
    )
    if kkey not in _program_cache:
        _program_cache[kkey] = _build_program(kkey)
    nc, lay = _program_cache[kkey]

    # ---- pack inputs ----
    xb = np.zeros((N_PAD, C), ml_dtypes.bfloat16)
    xb[:N_VERTS] = x.astype(ml_dtypes.bfloat16)
    x8 = np.ascontiguousarray(xb.reshape(NR8, R * C))

    iota = np.tile(np.arange(C, dtype=np.float32), lay.Kmax).astype(
        ml_dtypes.bfloat16
    )
    iota_img = np.broadcast_to(iota, (P, lay.Kmax * C))
    wt = np.ascontiguousarray(W.T.astype(ml_dtypes.bfloat16))  # [c, o]
    wt_img = np.zeros((P, C), ml_dtypes.bfloat16)
    wt_img[:, :] = wt
    bbT_img = np.ascontiguousarray(b[:, None].astype(np.float32))  # [o, 1]

    e_core = core_of_row[rows]
    tau_e_all = tloc_of_row[rows]
    d_e_all = slot_of_row[rows]
    r_e_all = (cols % R).astype(np.int32)
    idx_e_all = (cols // R).astype(np.int32)

    in_maps = []
    for c in range(N_CORES):
        sel = e_core == c
        idx_img, d_bf, v_bf = _pack_core(
            lay,
            tau_e_all[sel],
            r_e_all[sel],
            idx_e_all[sel],
            d_e_all[sel],
            vals[sel],
        )
        consts = np.concatenate(
            [
                np.ascontiguousarray(iota_img).view(np.int32),
                wt_img.view(np.int32),
                bbT_img.view(np.int32),
                d_bf.view(np.int32),
                v_bf.view(np.int32),
            ],
            axis=1,
        )
        in_maps.append(
            {"x8": x8, "idx": np.ascontiguousarray(idx_img), "consts": consts}
        )

    trace = bool(os.environ.get("BASS_KERNEL_TRACE"))
    res = run_bass_kernel_spmd(nc, in_maps, list(range(N_CORES)), trace=trace)
    LAST_EXEC_NS = getattr(res, "exec_time_ns", None)
    LAST_MEAN_EXEC_NS = getattr(res, "mean_exec_time_ns", None)

    outs = [
        np.asarray(res.results[i]["out"])  # [TPC//4, C, 4*P] transposed tiles
        .reshape(TPC // 4, C, 4, P)
        .transpose(0, 2, 3, 1)
        .reshape(TPC * P, C)
        for i in range(N_CORES)
    ]
    full = np.concatenate(outs, axis=0)  # [N_PAD, C] in permuted order
    row_position = (
        core_of_row.astype(np.int64) * (TPC * P)
        + tloc_of_row.astype(np.int64) * P
        + slot_of_row
    )
    return np.ascontiguousarray(full[row_position[:N_VERTS]], dtype=np.float32)

